# revision 1
# baseline (speedup 1.0000x reference)
"""GuidedAttentionLoss on 8 Trainium2 NeuronCores (Bass/Tile).

loss = mean(guide * a^T) over [B=64, T=2048, N=512], where
  guide[b,t,k] = (1 - exp(-((k - floor(N_b/T_b * t))/N_b)^2 / (2*sigma^2)))
                 for t < T_b, k < N_b; 0 elsewhere.

Strategy (pure data parallel, 8 batches per core), v3:
  * Key identity: o_t = floor(N_b/T_b * t) takes only N_b distinct values,
    each over a run of consecutive t's (run lengths ~T/N).  So
      sum_t (e[k,o_t]-1) a[k,t] = sum_{o,j} (e[k,o]-1) P[k, j, o]
    where P[k, j, o] = a[k, t(o,j)] is a host-side column permutation of a
    into W "slabs" of width N (zero padded).  exp work shrinks by ~T/N.
  * e[k,o] = exp(-c_b (k-o)^2) computed by ScalarE directly from a constant
    integer (k-o)^2 table (bf16) with per-partition AP scale = -c_b.  No
    matmul, no PE, no fp32 4-cycles/row penalty, no cancellation.
  * a is staged as float8_e4m3 or bfloat16 per slot (loss tol 2e-2; RTN
    quantization noise averages out over 38M elements).  DVE runs the fused
    (e-1)*P scalar_tensor_tensor at 2x (all-SBUF) for fp8 and 4x (2-byte
    packed + all-SBUF) for bf16; DMA is 1B/elem for fp8, 2B for bf16.  The
    fp8/bf16 split is chosen to balance DVE time against DMA time.
  * stt per (slot, row-tile): in0 = e row broadcast W times via stride-0 AP
    (3D, BIR limit), accum_out per unit; host sums valid lanes in f64.
  * DMAs are row-trimmed ([rows, W*N], skipping zero rows of the last row
    tile); garbage in the untouched partitions stays lane-isolated in the
    accumulator and is masked out on the host.
  * 64 batches dealt into 8 slots x 8 cores by simulated annealing + sweep
    minimizing sum ntiles*W*N (DVE cols, also ~DMA bytes).
"""

import os

import numpy as np
import ml_dtypes

# experiment toggles (harness uses defaults)
_FORCE_MASK = os.environ.get("K_FORCE_MASK")  # int: per-slot bf16 bitmask
# fp8 in DRAM upcast to bf16 by the (SWDGE) DMA. Gives 4x DVE mode on all
# slots, but the DMA transfer is charged at the 2-byte output side, so it
# measured ~33-40us vs ~23us for the staged mix.  Default OFF.
_CAST_DMA = os.environ.get("K_CAST_DMA", "0") == "1"
# split a slot's cast DMA in two (full tiles + row-trimmed last tile) when
# the last row tile has more than this many bytes of zero-row padding.
_TRIM_THRESH = int(os.environ.get("K_TRIM_THRESH", "65536"))

B, N_MAX, T_MAX = 64, 512, 2048
SIGMA = 0.4
N_CORES = 8
PART = 128
NTILES_MAX = 4  # ceil(N_MAX / PART)
F8 = ml_dtypes.float8_e4m3
BF16 = ml_dtypes.bfloat16

# engine model (per core): ns per free-dim column / per byte
_DVE_NS = 1.0416666  # 1x; 2x all-SBUF, 4x 2-byte packed all-SBUF
_ACT_NS = 0.8333333
_DMA_BPNS = 360.0  # bytes per ns, all 16 engines


def _runs(Nb, Tb):
    """Per-t offset o_t (exact reference fp32 math), slab index j_t, W."""
    t = np.arange(Tb, dtype=np.float32)
    ratio = np.float32(Nb) / np.float32(Tb)
    o = np.floor(ratio * t).astype(np.int64)
    starts = np.empty(Tb, dtype=bool)
    starts[0] = True
    starts[1:] = o[1:] != o[:-1]
    first = np.zeros(int(o[-1]) + 1, dtype=np.int64)
    first[o[starts]] = np.nonzero(starts)[0]
    j = np.arange(Tb, dtype=np.int64) - first[o]
    return o, j, int(j.max()) + 1


def _plan(input_lengths, target_lengths):
    """Assign batches to (slot, core); pick per-slot dtype (fp8/bf16).

    Returns list of slot dicts: idxs, N_s, W_s, ntiles, free, base, bf16.
    base indexes into the dtype's own blob.
    """
    Ns = np.asarray(input_lengths, dtype=np.int64)
    Ts = np.asarray(target_lengths, dtype=np.int64)
    assert Ns.shape == (B,) and Ts.shape == (B,)
    Ws = np.array([_runs(int(Ns[b]), int(Ts[b]))[2] for b in range(B)],
                  dtype=np.int64)
    n_slots = B // N_CORES

    def slot_cost(g):
        N_s = int(max(Ns[i] for i in g))
        W_s = int(max(Ws[i] for i in g))
        return (-(-N_s // PART)) * W_s * N_s  # DVE free-dim columns

    rng = np.random.default_rng(0)

    def sweep(groups):
        improved = True
        while improved:
            improved = False
            for s1 in range(n_slots):
                for s2 in range(s1 + 1, n_slots):
                    g1, g2 = groups[s1], groups[s2]
                    for i1 in range(N_CORES):
                        for i2 in range(N_CORES):
                            c0 = slot_cost(g1) + slot_cost(g2)
                            g1[i1], g2[i2] = g2[i2], g1[i1]
                            if slot_cost(g1) + slot_cost(g2) < c0:
                                improved = True
                            else:
                                g1[i1], g2[i2] = g2[i2], g1[i1]
        return groups

    def anneal(groups, iters=120000, T0=400.0, T1=0.5):
        groups = [list(g) for g in groups]
        costs = [slot_cost(g) for g in groups]
        cur = sum(costs)
        best, bestg = cur, [list(g) for g in groups]
        log_ratio = np.log(T1 / T0)
        u_rand = rng.random(iters)
        idx = rng.integers(0, 8, size=(iters, 4))
        for it in range(iters):
            s1, s2, i1, i2 = idx[it]
            if s1 == s2:
                continue
            T = T0 * np.exp(log_ratio * it / iters)
            g1, g2 = groups[s1], groups[s2]
            g1[i1], g2[i2] = g2[i2], g1[i1]
            c1, c2 = slot_cost(g1), slot_cost(g2)
            d = c1 + c2 - costs[s1] - costs[s2]
            if d <= 0 or u_rand[it] < np.exp(-d / T):
                costs[s1], costs[s2] = c1, c2
                cur += d
                if cur < best:
                    best, bestg = cur, [list(g) for g in groups]
            else:
                g1[i1], g2[i2] = g2[i2], g1[i1]
        return best, bestg

    order = np.argsort(-(Ws * 10000 + Ns))
    g0 = [list(order[s * N_CORES: (s + 1) * N_CORES]) for s in range(n_slots)]
    best_cost, best_g = anneal(g0, iters=250000)
    for _ in range(2):
        perm = rng.permutation(B)
        c, g = anneal([list(perm[s * N_CORES: (s + 1) * N_CORES])
                       for s in range(n_slots)], iters=150000)
        if c < best_cost:
            best_cost, best_g = c, g
    best_g = sweep([list(g) for g in best_g])

    raw = []
    for g in best_g:
        idxs = np.array([int(i) for i in g])
        N_s = int(Ns[idxs].max())
        W_s = int(Ws[idxs].max())
        ntiles = -(-N_s // PART)
        raw.append(dict(idxs=idxs, N_s=N_s, W_s=W_s, ntiles=ntiles,
                        free=ntiles * W_s * N_s))

    # dtype split: brute-force the subset of bf16 slots that minimizes
    # max(DVE, DMA) under the cost model.
    v = [sl["ntiles"] * sl["W_s"] * sl["N_s"] for sl in raw]  # DVE cols
    dbytes = [sl["N_s"] * sl["W_s"] * sl["N_s"] for sl in raw]  # trimmed B
    act = sum(sl["ntiles"] * sl["N_s"] for sl in raw) * _ACT_NS
    best_t, best_mask = None, 0
    for mask in range(1 << n_slots):
        dve = sum(v[s] * (_DVE_NS * (0.25 if mask >> s & 1 else 0.5))
                  for s in range(n_slots))
        dma = sum(dbytes[s] * (2 if mask >> s & 1 else 1)
                  for s in range(n_slots)) / _DMA_BPNS
        t = max(dve, dma, act)
        if best_t is None or t < best_t:
            best_t, best_mask = t, mask
    if _FORCE_MASK is not None:
        best_mask = int(_FORCE_MASK)
    if _CAST_DMA:
        best_mask = 0  # stage everything fp8; compute upcasts via DMA

    for s, sl in enumerate(raw):
        sl["bf16"] = bool(best_mask >> s & 1)
    # Interleave bf16 (DMA-heavy) and fp8 (DVE-heavy) slots so neither
    # engine's queue drains while the other backs up; largest first within
    # each class so the tail is short.
    g16 = sorted([sl for sl in raw if sl["bf16"]],
                 key=lambda sl: -sl["free"])
    g8 = sorted([sl for sl in raw if not sl["bf16"]],
                key=lambda sl: -sl["free"])
    order, err = [], 0
    i8 = i16 = 0
    for _ in range(len(raw)):
        err += len(g16)
        if i16 < len(g16) and (err >= len(raw) or i8 >= len(g8)):
            order.append(g16[i16])
            i16 += 1
            err -= len(raw)
        else:
            order.append(g8[i8])
            i8 += 1
    slots, base8, base16 = [], 0, 0
    for sl in order:
        sl["base"] = base16 if sl["bf16"] else base8
        if sl["bf16"]:
            base16 += sl["free"]
        else:
            base8 += sl["free"]
        slots.append(sl)
    return slots, base8, base16


def _host_inputs(alignments, input_lengths, target_lengths, slots,
                 len8, len16):
    """Per-core input dicts for run_bass_kernel_spmd."""
    alignments = np.asarray(alignments)
    n_slots = len(slots)

    # Constant (k-o)^2 table, shared by all cores: D2[p, rt*N_MAX + o].
    p = np.arange(PART, dtype=np.float32)[:, None]
    o = np.arange(N_MAX, dtype=np.float32)[None, :]
    d2 = np.concatenate(
        [((rt * PART + p) - o) ** 2 for rt in range(NTILES_MAX)], axis=1
    ).astype(BF16)

    in_maps = []
    for core in range(N_CORES):
        blob8 = np.zeros((PART, max(len8, 1)), dtype=F8)
        blob16 = np.zeros((PART, max(len16, 1)), dtype=BF16)
        scales = np.zeros((PART, n_slots), dtype=np.float32)
        for s, sl in enumerate(slots):
            b = int(sl["idxs"][core])
            Nb = int(input_lengths[b])
            Tb = int(target_lengths[b])
            N_s, W_s = sl["N_s"], sl["W_s"]
            o_t, j_t, _ = _runs(Nb, Tb)
            cols = j_t * N_s + o_t  # within a row tile
            blob = blob16 if sl["bf16"] else blob8
            a_cast = alignments[b, :Nb, :Tb].astype(blob.dtype)
            for rt in range(sl["ntiles"]):
                lo = rt * PART
                hi = min(lo + PART, Nb)
                if hi <= lo:
                    break
                blob[0: hi - lo, sl["base"] + rt * (W_s * N_s) + cols] = \
                    a_cast[lo:hi]
            scales[:, s] = np.float32(-1.0) / np.float32(
                2.0 * SIGMA * SIGMA * Nb * Nb
            )
        in_maps.append({"blob8": blob8, "blob16": blob16, "d2": d2,
                        "scales": scales})
    return in_maps


def _build_bass(slots, reps: int = 1):
    import concourse.bacc as bacc
    import concourse.mybir as mybir
    from concourse.tile import TileContext

    fp32 = mybir.dt.float32
    bf16 = mybir.dt.bfloat16
    f8 = mybir.dt.float8e4
    n_slots = len(slots)
    len8 = sum(sl["free"] for sl in slots if not sl["bf16"])
    len16 = sum(sl["free"] for sl in slots if sl["bf16"])
    max_f8 = max([sl["free"] for sl in slots if not sl["bf16"]] or [1])
    max_f16 = max([sl["free"] for sl in slots if sl["bf16"]] or [1])
    max_e = max(sl["ntiles"] * sl["N_s"] for sl in slots)
    max_m = max(sl["W_s"] * sl["N_s"] for sl in slots)
    n_units = sum(sl["ntiles"] for sl in slots)

    nc = bacc.Bacc(
        "TRN2", target_bir_lowering=False, debug=False, num_devices=N_CORES
    )
    blob8_d = nc.dram_tensor("blob8", [PART, max(len8, 1)], f8,
                             kind="ExternalInput")
    blob16_d = nc.dram_tensor("blob16", [PART, max(len16, 1)], bf16,
                              kind="ExternalInput")
    d2_d = nc.dram_tensor("d2", [PART, NTILES_MAX * N_MAX], bf16,
                          kind="ExternalInput")
    sc_d = nc.dram_tensor("scales", [PART, n_slots], fp32, kind="ExternalInput")
    oacc_d = nc.dram_tensor("out_acc", [PART, n_units], fp32,
                            kind="ExternalOutput")

    with TileContext(nc) as tc:
        with (
            tc.tile_pool(name="const", bufs=1) as constp,
            tc.tile_pool(name="blob8p", bufs=5) as blob8p,
            tc.tile_pool(name="blob16p", bufs=5) as blob16p,
            tc.tile_pool(name="epool", bufs=4) as epool,
            tc.tile_pool(name="mpool", bufs=4) as mpool,
            tc.tile_pool(name="accp", bufs=1) as accp,
        ):
            d2_sb = constp.tile([PART, NTILES_MAX * N_MAX], bf16, tag="d2")
            nc.sync.dma_start(out=d2_sb[:], in_=d2_d.ap()[:])
            sc_sb = constp.tile([PART, n_slots], fp32, tag="sc")
            nc.sync.dma_start(out=sc_sb[:], in_=sc_d.ap()[:])
            acc = accp.tile([PART, n_units], fp32, tag="acc")
            nc.vector.memset(acc[:], 0.0)

            for _rep in range(reps):
                u = 0
                for s, sl in enumerate(slots):
                    nt, W, N = sl["ntiles"], sl["W_s"], sl["N_s"]
                    free = sl["free"]
                    if _CAST_DMA:
                        # fp8 in DRAM, upcast to bf16 during the (SWDGE) DMA
                        blob_t = blob8p.tile([PART, max_f8], bf16, tag="b8")
                        rows_last = N - (nt - 1) * PART
                        pad = (PART - rows_last) * W * N
                        if pad > _TRIM_THRESH and nt > 1:
                            split = (nt - 1) * W * N
                            nc.gpsimd.dma_start(
                                out=blob_t[:, :split],
                                in_=blob8_d.ap()[:, sl["base"]:
                                                 sl["base"] + split],
                            )
                            nc.gpsimd.dma_start(
                                out=blob_t[0:rows_last, split:free],
                                in_=blob8_d.ap()[0:rows_last,
                                                 sl["base"] + split:
                                                 sl["base"] + free],
                            )
                        else:
                            nc.gpsimd.dma_start(
                                out=blob_t[:, :free],
                                in_=blob8_d.ap()[:, sl["base"]:
                                                 sl["base"] + free],
                            )
                    else:
                        if sl["bf16"]:
                            blob_t = blob16p.tile([PART, max_f16], bf16,
                                                  tag="b16")
                            blob_src = blob16_d
                        else:
                            blob_t = blob8p.tile([PART, max_f8], f8, tag="b8")
                            blob_src = blob8_d
                        for rt in range(nt):
                            rows = min(PART, N - rt * PART)
                            nc.sync.dma_start(
                                out=blob_t[0:rows,
                                           rt * W * N: (rt + 1) * W * N],
                                in_=blob_src.ap()[
                                    0:rows,
                                    sl["base"] + rt * W * N:
                                    sl["base"] + (rt + 1) * W * N,
                                ],
                            )
                    e_t = epool.tile([PART, max_e], bf16, tag="e")
                    d2_in = d2_sb[:, : nt * N_MAX].rearrange(
                        "p (r o) -> p r o", r=nt
                    )[:, :, 0:N]
                    e_out = e_t[:, : nt * N].rearrange("p (r o) -> p r o", r=nt)
                    nc.scalar.activation(
                        e_out, d2_in, mybir.ActivationFunctionType.Exp,
                        bias=0.0, scale=sc_sb[:, s: s + 1],
                    )
                    for rt in range(nt):
                        m_t = mpool.tile([PART, max_m], bf16, tag="m")
                        in0 = (
                            e_t[:, rt * N: (rt + 1) * N]
                            .unsqueeze(1)
                            .broadcast_to([PART, W, N])
                        )
                        in1 = blob_t[:, rt * W * N: (rt + 1) * W * N].rearrange(
                            "p (w o) -> p w o", w=W
                        )
                        m_out = m_t[:, : W * N].rearrange(
                            "p (w o) -> p w o", w=W
                        )
                        nc.vector.scalar_tensor_tensor(
                            out=m_out, in0=in0, scalar=1.0, in1=in1,
                            op0=mybir.AluOpType.subtract,
                            op1=mybir.AluOpType.mult,
                            accum_out=acc[:, u: u + 1],
                        )
                        u += 1
            nc.sync.dma_start(out=oacc_d.ap()[:], in_=acc[:])

    nc.compile()
    return nc


def _reduce_outputs(results, slots):
    tot = 0.0
    for res in results:
        acc = np.asarray(res["out_acc"], dtype=np.float64)
        u = 0
        for sl in slots:
            N = sl["N_s"]
            for rt in range(sl["ntiles"]):
                rows = min(PART, N - rt * PART)
                tot += acc[0:rows, u].sum()
                u += 1
    loss = -tot / float(B * N_MAX * T_MAX)
    return np.array(loss, dtype=np.float32)


def kernel(alignments, input_lengths, target_lengths):
    from concourse.bass_utils import run_bass_kernel_spmd

    slots, len8, len16 = _plan(input_lengths, target_lengths)
    in_maps = _host_inputs(alignments, input_lengths, target_lengths, slots,
                           len8, len16)
    nc = _build_bass(slots, reps=1)
    out = run_bass_kernel_spmd(nc, in_maps, core_ids=list(range(N_CORES)))
    return _reduce_outputs(out.results, slots)


if __name__ == "__main__":
    rng = np.random.default_rng(0)
    al = rng.random((B, N_MAX, T_MAX), dtype=np.float32)
    il = rng.integers(N_MAX // 2, N_MAX + 1, size=B).astype(np.int32)
    tl = rng.integers(T_MAX // 2, T_MAX + 1, size=B).astype(np.int32)
    print(kernel(alignments=al, input_lengths=il, target_lengths=tl))



# revision 5
# speedup vs baseline: 4.7332x; 4.7332x over previous
"""GuidedAttentionLoss on 8 Trainium2 NeuronCores (Bass/Tile), v4: PE matmul.

loss = mean(guide * a^T) over [B=64, T=2048, N=512], where
  guide[b,t,k] = G_b[k, o_t],  G_b[k,o] = 1 - exp(-((k-o)/N_b)^2/(2 s^2)),
  o_t = floor(N_b/T_b * t), valid for t < T_b, k < N_b.

Key identity: G_b[k,o] = phi((k-o)/N_b) is a smooth Gaussian-type kernel on
a bounded domain, so it is numerically low rank: G_b ~= U_b V_b^T with
R ~ 10 (sigma_11/sigma_1 ~ 1e-10).  Then

  term_b = sum_{k,t} G_b[k,o_t] a[k,t]
         = sum_r sum_k U_b[k,r] * (sum_t V_b[o_t,r] a[k,t])
         = <U_b^T, Vt_b^T A_b^T>   with  Vt_b[t,r] = V_b[o_t,r].

The inner contraction over t is a PE matmul Z = Vt^T @ A^T (contract t on
partitions, 128 per step, fp8 DoubleRow = 256 per step at 0.5 cyc/row),
accumulated in PSUM [R, N_b].  The outer <U, Z> is one tiny DVE
scalar_tensor_tensor with accum_out per batch.  No exp on device at all.

Per core (8 batches, one per slot):
  * a^T staged fp8 [128(t), SUBT, N_s]; Vt staged fp8 [128(t), SUBT, R]
    (host-expanded V[o_t], zero rows for t >= T_b mask the t padding;
    zero U columns for k >= N_b mask the k padding).
  * V columns are pow2-scaled to ~[64,128) max-abs for fp8; U is refit
    against the quantized V by least squares on host (kills quantization
    bias), applied in bf16 on the DVE.
  * Engine model: DMA ~ sum 128*SUBT*N_s bytes ~ 5MB -> ~14us (bound);
    PE ~ sum SUBT/2*N_s*0.5cyc ~ 7-12us; DVE ~ sum N_s ~ 4us; ACT 0.
  * 64 batches dealt into 8 slots x 8 cores by annealing + sweep on
    cost = SUBT_s * N_s (DMA bytes = PE work).
"""

import numpy as np
import ml_dtypes

B, N_MAX, T_MAX = 64, 512, 2048
SIGMA = 0.4
N_CORES = 8
PART = 128
R = 16  # rank of the guide factorization (16 = fp8 DoubleRow
        # LdWeights step alignment; rank 10 already exact)
F8 = ml_dtypes.float8_e4m3
BF16 = ml_dtypes.bfloat16

# engine model (per core): ns per unit
_PE_NS = 0.8333  # mid p-state cycle; full speed is 0.4167
_DVE_NS = 1.0416666
_DMA_BPNS = 360.0


def _offsets(Nb, Tb):
    """Per-t offset o_t with exact reference fp32 math."""
    t = np.arange(Tb, dtype=np.float32)
    ratio = np.float32(Nb) / np.float32(Tb)
    return np.floor(ratio * t).astype(np.int64)


_factor_cache: dict[tuple[int, int], tuple[np.ndarray, np.ndarray]] = {}


def _factors(Nb):
    """Low-rank factors of G[k,o] = 1 - exp(-((k-o)/Nb)^2/(2 sigma^2)).

    Returns (Ut [R, Nb] float32 refit, Vq [Nb, R] float8).  V columns are
    pow2-scaled into fp8 range; U is the least-squares refit of G against
    the quantized V, so fp8 quantization of V adds no bias.
    """
    key = (Nb, R)
    if key in _factor_cache:
        return _factor_cache[key]
    k = np.arange(Nb, dtype=np.float64)
    G = 1.0 - np.exp(-np.subtract.outer(k, k) ** 2 /
                     (2.0 * SIGMA * SIGMA * Nb * Nb))
    _, _, Vt_ = np.linalg.svd(G)
    r = min(R, Nb)
    V = Vt_[:r].T  # [Nb, r], unit columns
    sc = 2.0 ** np.floor(np.log2(64.0 / np.abs(V).max(axis=0)))
    Vq = (V * sc).astype(F8)
    Vd = Vq.astype(np.float64)
    Ut, *_ = np.linalg.lstsq(Vd.T @ Vd, Vd.T @ G.T, rcond=None)  # [r, Nb]
    if r < R:
        Ut = np.concatenate([Ut, np.zeros((R - r, Nb))], axis=0)
        Vq = np.concatenate([Vq, np.zeros((Nb, R - r), dtype=F8)], axis=1)
    out = (Ut.astype(np.float32), Vq)
    _factor_cache[key] = out
    return out


def _plan(input_lengths, target_lengths):
    """Assign batches to (slot, core) minimizing sum_s SUBT_s * N_s.

    Returns (slots, len_a, len_v, len_u); slot dicts have idxs, N_s, SUBT,
    base_a, base_v, base_u.
    """
    Ns = np.asarray(input_lengths, dtype=np.int64)
    Ts = np.asarray(target_lengths, dtype=np.int64)
    assert Ns.shape == (B,) and Ts.shape == (B,)
    SUBTs = -(-Ts // PART)
    n_slots = B // N_CORES

    def slot_cost(g):
        return int(max(SUBTs[i] for i in g)) * int(max(Ns[i] for i in g))

    rng = np.random.default_rng(0)

    def sweep(groups):
        improved = True
        while improved:
            improved = False
            for s1 in range(n_slots):
                for s2 in range(s1 + 1, n_slots):
                    g1, g2 = groups[s1], groups[s2]
                    for i1 in range(N_CORES):
                        for i2 in range(N_CORES):
                            c0 = slot_cost(g1) + slot_cost(g2)
                            g1[i1], g2[i2] = g2[i2], g1[i1]
                            if slot_cost(g1) + slot_cost(g2) < c0:
                                improved = True
                            else:
                                g1[i1], g2[i2] = g2[i2], g1[i1]
        return groups

    def anneal(groups, iters=150000, T0=400.0, T1=0.5):
        groups = [list(g) for g in groups]
        costs = [slot_cost(g) for g in groups]
        cur = sum(costs)
        best, bestg = cur, [list(g) for g in groups]
        log_ratio = np.log(T1 / T0)
        u_rand = rng.random(iters)
        idx = rng.integers(0, 8, size=(iters, 4))
        for it in range(iters):
            s1, s2, i1, i2 = idx[it]
            if s1 == s2:
                continue
            T = T0 * np.exp(log_ratio * it / iters)
            g1, g2 = groups[s1], groups[s2]
            g1[i1], g2[i2] = g2[i2], g1[i1]
            c1, c2 = slot_cost(g1), slot_cost(g2)
            d = c1 + c2 - costs[s1] - costs[s2]
            if d <= 0 or u_rand[it] < np.exp(-d / T):
                costs[s1], costs[s2] = c1, c2
                cur += d
                if cur < best:
                    best, bestg = cur, [list(g) for g in groups]
            else:
                g1[i1], g2[i2] = g2[i2], g1[i1]
        return best, bestg

    order = np.argsort(-(SUBTs * 10000 + Ns))
    g0 = [list(order[s * N_CORES: (s + 1) * N_CORES]) for s in range(n_slots)]
    best_cost, best_g = anneal(g0, iters=200000)
    for _ in range(2):
        perm = rng.permutation(B)
        c, g = anneal([list(perm[s * N_CORES: (s + 1) * N_CORES])
                       for s in range(n_slots)], iters=120000)
        if c < best_cost:
            best_cost, best_g = c, g
    best_g = sweep([list(g) for g in best_g])

    # big slots first so the pipeline tail is short
    best_g.sort(key=lambda g: -slot_cost(g))
    slots, base_a, base_v, base_u = [], 0, 0, 0
    for g in best_g:
        idxs = np.array([int(i) for i in g])
        N_s = -(-int(Ns[idxs].max()) // 16) * 16  # DoubleRow pair-step % 16
        SUBT = int(SUBTs[idxs].max())
        slots.append(dict(idxs=idxs, N_s=N_s, SUBT=SUBT,
                          base_a=base_a, base_v=base_v, base_u=base_u))
        base_a += SUBT * N_s
        base_v += SUBT * R
        base_u += N_s
    return slots, base_a, base_v, base_u


def _host_inputs(alignments, input_lengths, target_lengths, slots,
                 len_a, len_v, len_u):
    """Per-core input dicts for run_bass_kernel_spmd."""
    alignments = np.asarray(alignments)
    in_maps = []
    for core in range(N_CORES):
        blob_a = np.zeros((PART, len_a), dtype=F8)
        blob_v = np.zeros((PART, len_v), dtype=F8)
        blob_u = np.zeros((R, len_u), dtype=BF16)
        for sl in slots:
            b = int(sl["idxs"][core])
            Nb = int(input_lengths[b])
            Tb = int(target_lengths[b])
            N_s, SUBT = sl["N_s"], sl["SUBT"]
            Ut, Vq = _factors(Nb)
            o_t = _offsets(Nb, Tb)

            aT = np.ascontiguousarray(
                alignments[b, :Nb, :Tb].T).astype(F8)  # [Tb, Nb]
            a3 = blob_a[:, sl["base_a"]: sl["base_a"] + SUBT * N_s]
            a3 = a3.reshape(PART, SUBT, N_s)
            full, rem = Tb // PART, Tb % PART
            if full:
                a3[:, :full, :Nb] = aT[: full * PART].reshape(
                    full, PART, Nb).transpose(1, 0, 2)
            if rem:
                a3[:rem, full, :Nb] = aT[full * PART:]

            Vexp = Vq[o_t]  # [Tb, R] fp8
            v3 = blob_v[:, sl["base_v"]: sl["base_v"] + SUBT * R]
            v3 = v3.reshape(PART, SUBT, R)
            if full:
                v3[:, :full, :] = Vexp[: full * PART].reshape(
                    full, PART, R).transpose(1, 0, 2)
            if rem:
                v3[:rem, full, :] = Vexp[full * PART:]

            blob_u[:, sl["base_u"]: sl["base_u"] + Nb] = Ut.astype(BF16)
        in_maps.append({"blob_a": blob_a, "blob_v": blob_v, "blob_u": blob_u})
    return in_maps


def _build_bass(slots, reps: int = 1):
    import concourse.bacc as bacc
    import concourse.mybir as mybir
    from concourse.tile import TileContext

    fp32 = mybir.dt.float32
    bf16 = mybir.dt.bfloat16
    f8 = mybir.dt.float8e4
    n_slots = len(slots)
    len_a = sum(sl["SUBT"] * sl["N_s"] for sl in slots)
    len_v = sum(sl["SUBT"] * R for sl in slots)
    len_u = sum(sl["N_s"] for sl in slots)
    max_a = max(sl["SUBT"] * sl["N_s"] for sl in slots)
    max_n = max(sl["N_s"] for sl in slots)

    nc = bacc.Bacc(
        "TRN2", target_bir_lowering=False, debug=False, num_devices=N_CORES
    )
    a_d = nc.dram_tensor("blob_a", [PART, len_a], f8, kind="ExternalInput")
    v_d = nc.dram_tensor("blob_v", [PART, len_v], f8, kind="ExternalInput")
    u_d = nc.dram_tensor("blob_u", [R, len_u], bf16, kind="ExternalInput")
    oacc_d = nc.dram_tensor("out_acc", [R, n_slots], fp32,
                            kind="ExternalOutput")

    with TileContext(nc) as tc:
        with (
            tc.tile_pool(name="const", bufs=1) as constp,
            tc.tile_pool(name="apool", bufs=3) as apool,
            tc.tile_pool(name="mpool", bufs=2) as mpool,
            tc.psum_pool(name="zpool", bufs=4) as zpool,
        ):
            v_sb = constp.tile([PART, len_v], f8, tag="v")
            nc.sync.dma_start(out=v_sb[:], in_=v_d.ap()[:])
            u_sb = constp.tile([R, len_u], bf16, tag="u")
            nc.sync.dma_start(out=u_sb[:], in_=u_d.ap()[:])
            acc = constp.tile([R, n_slots], fp32, tag="acc")
            nc.vector.memset(acc[:], 0.0)

            for _rep in range(reps):
                for s, sl in enumerate(slots):
                    SUBT, N_s = sl["SUBT"], sl["N_s"]
                    a_t = apool.tile([PART, max_a], f8, tag="a")
                    a3 = a_t[:, : SUBT * N_s].rearrange(
                        "p (j n) -> p j n", j=SUBT)
                    # chunk the slot DMA at even-subtile boundaries so the
                    # first DoubleRow pairs can start as soon as their
                    # chunk lands, and chunks spread across DMA queues
                    sp = 0
                    while sp < SUBT:
                        ep = min(sp + 4, SUBT)
                        nc.sync.dma_start(
                            out=a_t[:, sp * N_s: ep * N_s],
                            in_=a_d.ap()[:, sl["base_a"] + sp * N_s:
                                         sl["base_a"] + ep * N_s],
                        )
                        sp = ep
                    v3 = v_sb[:, sl["base_v"]: sl["base_v"] + SUBT * R
                              ].rearrange("p (j r) -> p j r", j=SUBT)
                    z_t = zpool.tile([R, max_n], fp32, tag="z")
                    npairs = SUBT // 2
                    for i in range(npairs):
                        nc.tensor.matmul(
                            z_t[:, :N_s],
                            v3[:, 2 * i: 2 * i + 2, :],
                            a3[:, 2 * i: 2 * i + 2, :],
                            start=(i == 0),
                            stop=(i == npairs - 1 and SUBT % 2 == 0),
                            perf_mode=mybir.MatmulPerfMode.DoubleRow,
                        )
                    if SUBT % 2:
                        nc.tensor.matmul(
                            z_t[:, :N_s],
                            v3[:, SUBT - 1: SUBT, :],
                            a3[:, SUBT - 1: SUBT, :],
                            start=(npairs == 0),
                            stop=True,
                        )
                    m_t = mpool.tile([R, max_n], bf16, tag="m")
                    nc.vector.scalar_tensor_tensor(
                        out=m_t[:, :N_s],
                        in0=z_t[:, :N_s],
                        scalar=1.0,
                        in1=u_sb[:, sl["base_u"]: sl["base_u"] + N_s],
                        op0=mybir.AluOpType.mult,
                        op1=mybir.AluOpType.mult,
                        accum_out=acc[:, s: s + 1],
                    )
            nc.sync.dma_start(out=oacc_d.ap()[:], in_=acc[:])

    nc.compile()
    return nc


def _reduce_outputs(results):
    tot = 0.0
    for res in results:
        tot += np.asarray(res["out_acc"], dtype=np.float64).sum()
    return np.array(tot / float(B * N_MAX * T_MAX), dtype=np.float32)


def kernel(alignments, input_lengths, target_lengths):
    from concourse.bass_utils import run_bass_kernel_spmd

    slots, len_a, len_v, len_u = _plan(input_lengths, target_lengths)
    in_maps = _host_inputs(alignments, input_lengths, target_lengths, slots,
                           len_a, len_v, len_u)
    nc = _build_bass(slots, reps=1)
    out = run_bass_kernel_spmd(nc, in_maps, core_ids=list(range(N_CORES)))
    return _reduce_outputs(out.results)


if __name__ == "__main__":
    rng = np.random.default_rng(0)
    al = rng.random((B, N_MAX, T_MAX), dtype=np.float32)
    il = rng.integers(N_MAX // 2, N_MAX + 1, size=B).astype(np.int32)
    tl = rng.integers(T_MAX // 2, T_MAX + 1, size=B).astype(np.int32)
    print(kernel(alignments=al, input_lengths=il, target_lengths=tl))
